# revision 1
# baseline (speedup 1.0000x reference)
"""Blockwise-parallel transformer layer on 8 TRN2 NeuronCores.

Sharding: sequence-parallel over q rows (256 rows/core), K/V projections
replicated on every core (cheaper than an intra-chip allgather at these
sizes).  All weights are pre-transposed host-side so every matmul operand
DMAs contiguously; matmuls run as float32r (exact fp32 bits, streamed at
full PE rate when the moving free dim >= 256).

Shapes (hardcoded):
  x (1, 2048, 1024); Wq/Wk/Wv (1024, 1024); W1 (4096, 1024); W2 (1024, 4096)
  H=16 heads * HD=64; KB=16 kv blocks of 128 (blockwise local-max softmax).
"""

import sys
from contextlib import ExitStack

import numpy as np

for _p in ("/opt/trn_rl_repo", "/root/.axon_site/_ro/trn_rl_repo"):
    if _p not in sys.path:
        sys.path.append(_p)

import concourse.bass as bass  # noqa: E402
import concourse.tile as tile  # noqa: E402
from concourse import bacc, mybir  # noqa: E402
from concourse._compat import with_exitstack  # noqa: E402
from concourse.bass import ds  # noqa: E402
from concourse.bass_utils import run_bass_kernel_spmd  # noqa: E402
from concourse.masks import make_identity  # noqa: E402

D = 1024
H = 16
HD = 64
FF = 4096
N = 2048
KB = 16
NCORES = 8
RQ = N // NCORES  # 256 q rows per core
P = 128

F32 = mybir.dt.float32
F32R = mybir.dt.float32r
AX = mybir.AxisListType
AF = mybir.ActivationFunctionType


@with_exitstack
def _tile_kernel(ctx: ExitStack, tc: tile.TileContext, io: dict):
    nc = tc.nc

    consts = ctx.enter_context(tc.tile_pool(name="consts", bufs=1))
    ident = consts.tile([P, P], F32)
    make_identity(nc, ident)
    identr = consts.tile([P, P], F32R)
    nc.scalar.activation(out=identr, in_=ident, func=AF.Copy)
    # IND[k, j] = 1 iff k == j//128 (j < 1024) — rank-1 -max broadcast
    tmpp = ctx.enter_context(tc.tile_pool(name="tmp_ind", bufs=1))
    indf = tmpp.tile([8, 8 * P], F32)
    nc.gpsimd.memset(indf, 1.0)
    # keep where j - 128k >= 0, else 0
    nc.gpsimd.affine_select(
        out=indf, in_=indf, compare_op=mybir.AluOpType.is_ge, fill=0.0,
        base=0, pattern=[[1, 8 * P]], channel_multiplier=-P,
    )
    # keep where 127 + 128k - j >= 0, else 0
    nc.gpsimd.affine_select(
        out=indf, in_=indf, compare_op=mybir.AluOpType.is_ge, fill=0.0,
        base=P - 1, pattern=[[-1, 8 * P]], channel_multiplier=P,
    )
    indc = consts.tile([8, 8 * P], F32R)
    nc.scalar.activation(out=indc, in_=indf, func=AF.Copy)
    bqs = consts.tile([P, 16], F32)
    bks = consts.tile([P, 8], F32)
    bvs = consts.tile([P, 8], F32)
    b1s = consts.tile([P, 32], F32)
    b2s = consts.tile([P, 8], F32)
    nc.sync.dma_start(out=bqs, in_=io["bq2d"])
    nc.sync.dma_start(out=bks, in_=io["bk2d"])
    nc.sync.dma_start(out=bvs, in_=io["bv2d"])
    nc.sync.dma_start(out=b1s, in_=io["b12d"])
    nc.sync.dma_start(out=b2s, in_=io["b22d"])

    persist = ctx.enter_context(tc.tile_pool(name="persist", bufs=1))
    attn = persist.tile([P, 2, D], F32)  # [q'%128, qtile, channel]

    with tc.tile_pool(name="kvq", bufs=1) as kvp:
        _attention_phases(tc, io, kvp, ident, identr, indc, bqs, bks, bvs, attn)

    # ---- Phase 3: FFN + residuals ---------------------------------------
    _ffn_phase(tc, io, ident, b1s, b2s, attn)


def _attention_phases(tc, io, kvp, ident, identr, indc, bqs, bks, bvs, attn):
    # Reference semantics ('Qhqd,Kkhd->QhqKk' with reshaped axes): for query
    # at seq pos n, the K/V channel slice is head i = n%16, while all 16
    # channel groups g of q are evaluated against it.  Queries are fed
    # host-permuted so group i occupies local rows i*16..i*16+16; qTp stores
    # q channels duplicated on both partition halves so the scores lhsT for
    # any i shares the kT slice's base partition.
    nc = tc.nc
    kT = kvp.tile([P, 8, N], F32R)  # k transposed: [d%128, d//128, kv pos]
    vT = kvp.tile([P, 8, N], F32R)

    # ---- Phase 1a: K/V projections (full sequence, replicated) ----------
    NCH = 4  # n-chunks of 512 columns
    CW = N // NCH
    for wname, bias_t, dst in (("wkT", bks, kT), ("wvT", bvs, vT)):
        with (
            tc.tile_pool(name=f"w_{wname}", bufs=1) as wp,
            tc.tile_pool(name=f"xs_{wname}", bufs=2) as xsp,
            tc.psum_pool(name=f"ps_{wname}", bufs=2) as pskv,
        ):
            w = wp.tile([P, 8, D], F32R, tag="w")  # [din%128, din//128, dout]
            for j in range(8):
                nc.sync.dma_start(out=w[:, j, :], in_=io[wname][ds(j * P, P), :])
            for c in range(NCH):
                # x chunk in two dx-halves to bound SBUF
                xcs = []
                for xh in range(2):
                    xc = xsp.tile([P, 4, CW], F32R, tag="xc")
                    for j in range(4):
                        nc.sync.dma_start(
                            out=xc[:, j, :],
                            in_=io["xT"][ds((xh * 4 + j) * P, P), ds(c * CW, CW)],
                        )
                    xcs.append(xc)
                for dk2 in range(4):  # dk pairs -> one [128,1024] psum
                    ps = pskv.tile([P, 2, CW], F32, tag="pkv")
                    for half in range(2):
                        for dx in range(8):
                            nc.tensor.matmul(
                                ps[:, half, :],
                                lhsT=w[:, dx, ds((dk2 * 2 + half) * P, P)],
                                rhs=xcs[dx // 4][:, dx % 4, :],
                                start=(dx == 0),
                                stop=(dx == 7),
                            )
                    for half in range(2):
                        dk = dk2 * 2 + half
                        nc.scalar.activation(
                            out=dst[:, dk, ds(c * CW, CW)],
                            in_=ps[:, half, :],
                            func=AF.Identity,
                            bias=bias_t[:, dk : dk + 1],
                        )

    # ---- Phase 1b: local Q projection (pre-scaled by 1/8 host-side) -----
    # wqT is host-duplicated: column gb*128+p holds Wq.T[:, gb*64 + p%64]/8,
    # so qTp[p, gb, :] = q[:, gb*64 + p%64] with both partition halves equal.
    qtp_pool = tc.tile_pool(name="qTp", bufs=1)
    qtp_ctx = qtp_pool.__enter__()
    # [(d dup on halves), kv-head i, g-half, g'*16+n''] — each (i, gh) block
    # is one contiguous 128-wide lhsT for the scores matmul.
    qTp = qtp_ctx.tile([P, 16, 2, P], F32R)
    with (
        tc.tile_pool(name="wq", bufs=1) as wqp,
        tc.tile_pool(name="xtl", bufs=1) as xtlp,
        tc.psum_pool(name="psq", bufs=3) as psq,
    ):
        xtl = xtlp.tile([P, 8, RQ], F32R)
        for j in range(8):
            nc.sync.dma_start(out=xtl[:, j, :], in_=io["xTloc"][ds(j * P, P), :])
        for gbh in range(2):  # gb halves of 8
            wq = wqp.tile([P, 8, 8 * P], F32R, tag="wq")
            for j in range(8):
                nc.sync.dma_start(
                    out=wq[:, j, :], in_=io["wqT"][ds(j * P, P), ds(gbh * 8 * P, 8 * P)]
                )
            for g in range(8):
                gb = gbh * 8 + g
                ps = psq.tile([P, RQ], F32, tag="pq")
                for dx in range(8):
                    nc.tensor.matmul(
                        ps,
                        lhsT=wq[:, dx, ds(g * P, P)],
                        rhs=xtl[:, dx, :],
                        start=(dx == 0),
                        stop=(dx == 7),
                    )
                nc.scalar.activation(
                    out=qTp[:, :, gb // 8, ds((gb % 8) * 16, 16)],
                    in_=ps.rearrange("p (i n) -> p i n", n=16),
                    func=AF.Identity,
                    bias=bqs[:, gb : gb + 1],
                )

    # ---- Phase 2: blockwise attention -----------------------------------
    # Per kv-head i (= query pos mod 16), per channel-group half gh:
    #   1) scores into one 4-bank psum tile s[128, 2048]
    #   2) one 3D reduce_max -> -max per kv block (nm)
    #   3) nm transposed once on PE; rank-1 matmul nmT.T @ IND adds -max
    #      broadcast to every score (exact-enough in f32r)
    #   4) single exp over [128, 2048], accum_out = den
    # then e-chunks are PE-transposed (packed x4) for the num matmuls.
    with (
        tc.tile_pool(name="att_e", bufs=2) as aep,
        tc.tile_pool(name="att_sm", bufs=3) as asp,
        tc.psum_pool(name="ps_s", bufs=1) as pss,
        tc.psum_pool(name="ps_et", bufs=2) as pse,
        tc.psum_pool(name="ps_tr", bufs=1) as pst,
        tc.psum_pool(name="ps_n", bufs=1) as psn,
    ):
        for i in range(16):
            c0, r0 = i // 2, (i % 2) * 64
            e = aep.tile([P, 2, N], F32R, tag="e")
            nm = asp.tile([P, 2, KB], F32, tag="nm")
            den = asp.tile([P, 2, 2], F32, tag="den")
            dsum = asp.tile([P, 2, 1], F32, tag="dsum")
            rcp = asp.tile([P, 2, 1], F32, tag="rcp")
            for gh in range(2):
                qsl = qTp[r0 : r0 + 64, i, gh, :]
                for sh in range(2):  # score halves [128, 1024]
                    s = pss.tile([P, 8 * P], F32, tag="s")
                    for c2 in range(2):
                        nc.tensor.matmul(
                            s[:, ds(c2 * 512, 512)],
                            lhsT=qsl,
                            rhs=kT[
                                r0 : r0 + 64, c0, ds(sh * 1024 + c2 * 512, 512)
                            ],
                            start=True,
                            stop=True,
                        )
                    nc.vector.reduce_max(
                        out=nm[:, gh, ds(sh * 8, 8)],
                        in_=s.rearrange("p (b f) -> p b f", f=P),
                        axis=AX.X,
                        negate=True,
                    )
                    nmt_ps = pst.tile([8, P], F32, tag="nmt")
                    nc.tensor.transpose(nmt_ps, nm[:, gh, ds(sh * 8, 8)], ident)
                    nmt = asp.tile([8, P], F32R, tag="nmt")
                    nc.vector.tensor_copy(out=nmt, in_=nmt_ps)
                    for c2 in range(2):
                        nc.tensor.matmul(
                            s[:, ds(c2 * 512, 512)],
                            lhsT=nmt,
                            rhs=indc[:, ds(c2 * 512, 512)],
                            start=False,
                            stop=True,
                            skip_group_check=True,
                        )
                    nc.scalar.activation(
                        out=e[:, gh, ds(sh * 1024, 1024)],
                        in_=s,
                        func=AF.Exp,
                        accum_out=den[:, gh, sh : sh + 1],
                    )
                nc.vector.reduce_sum(
                    out=dsum[:, gh, :], in_=den[:, gh, :], axis=AX.X
                )
                nc.vector.reciprocal(out=rcp[:, gh, :], in_=dsum[:, gh, :])

            nacc = psn.tile([64, RQ], F32, tag="nacc")
            for kc4 in range(4):  # kv-block quads
                vps = pst.tile([P, 4, 64], F32R, tag="tr")
                for k4 in range(4):
                    nc.tensor.transpose(
                        vps[:, k4, :],
                        vT[r0 : r0 + 64, c0, ds((kc4 * 4 + k4) * P, P)],
                        identr[r0 : r0 + 64, r0 : r0 + 64],
                    )
                vsb = asp.tile([P, 4, 64], F32R, tag="vsb")
                nc.vector.tensor_copy(out=vsb, in_=vps)
                for k2 in range(2):  # kc pairs -> one [128,512] etp
                    etp = pse.tile([P, 2, RQ], F32R, tag="etp")
                    for kk in range(2):
                        kc = kc4 * 4 + k2 * 2 + kk
                        for gh in range(2):
                            nc.tensor.transpose(
                                etp[:, kk, ds(gh * P, P)],
                                e[:, gh, ds(kc * P, P)],
                                identr,
                            )
                    ets = asp.tile([P, 2, RQ], F32R, tag="ets", bufs=2)
                    nc.scalar.activation(out=ets, in_=etp, func=AF.Copy)
                    for kk in range(2):
                        kc = kc4 * 4 + k2 * 2 + kk
                        nc.tensor.matmul(
                            nacc,
                            lhsT=vsb[:, kc % 4, :],
                            rhs=ets[:, kk, :],
                            start=(kc == 0),
                            stop=(kc == KB - 1),
                        )
            nsb = asp.tile([64, RQ], F32, tag="nsb", bufs=2)
            nc.vector.tensor_copy(out=nsb, in_=nacc)
            for gh in range(2):
                aps = pst.tile([P, 64], F32, tag="tr")
                nc.tensor.transpose(aps, nsb[:, ds(gh * P, P)], ident[0:64, 0:64])
                asb = asp.tile([P, 64], F32, tag="asb")
                nc.vector.tensor_scalar_mul(out=asb, in0=aps, scalar1=rcp[:, gh, :])
                # scatter (g', n'') partitions -> row' partitions + channels
                for gp in range(8):
                    nc.sync.dma_start(
                        out=attn[
                            ds((i % 8) * 16, 16),
                            i // 8,
                            ds((gh * 8 + gp) * HD, HD),
                        ],
                        in_=asb[ds(gp * 16, 16), :],
                    )
    qtp_pool.__exit__(None, None, None)


def _ffn_phase(tc, io, ident, b1s, b2s, attn):
    nc = tc.nc
    with (
        tc.tile_pool(name="ffn", bufs=1) as fp,
        tc.tile_pool(name="ffn_sm", bufs=3) as fsm,
        tc.tile_pool(name="wstream", bufs=2) as wsp,
    ):
        xl = fp.tile([P, 2, D], F32)  # local x rows (residual)
        nc.sync.dma_start(out=xl[:, 0, :], in_=io["xloc"][0:P, :])
        nc.sync.dma_start(out=xl[:, 1, :], in_=io["xloc"][P : 2 * P, :])
        h1 = fp.tile([P, 2, D], F32)
        for qt in range(2):
            nc.vector.tensor_add(out=h1[:, qt, :], in0=attn[:, qt, :], in1=xl[:, qt, :])
        h1T = fp.tile([P, 8, RQ], F32)
        h1Tr = fp.tile([P, 8, RQ], F32R)  # f32r copy for GEMM1 rhs
        with tc.psum_pool(name="ps_ft", bufs=2) as psft:
            for qt in range(2):
                for dc in range(8):
                    tps = psft.tile([P, P], F32, tag="tps")
                    nc.tensor.transpose(tps, h1[:, qt, ds(dc * P, P)], ident)
                    nc.vector.tensor_copy(out=h1T[:, dc, ds(qt * P, P)], in_=tps)
                    nc.scalar.activation(
                        out=h1Tr[:, dc, ds(qt * P, P)], in_=tps, func=AF.Copy
                    )
        hid = fp.tile([P, 32, RQ], F32R)
        with tc.psum_pool(name="ps_f", bufs=3) as psf:
            for q4 in range(4):  # W1 column quarters [128, 8, 1024]
                w1q = wsp.tile([P, 8, 8 * P], F32R, tag="wbig")
                for j in range(8):
                    nc.sync.dma_start(
                        out=w1q[:, j, :],
                        in_=io["w1T"][ds(j * P, P), ds(q4 * 8 * P, 8 * P)],
                    )
                for f in range(8):
                    ff = q4 * 8 + f
                    ps = psf.tile([P, RQ], F32, tag="fps")
                    for dc in range(8):
                        nc.tensor.matmul(
                            ps,
                            lhsT=w1q[:, dc, ds(f * P, P)],
                            rhs=h1Tr[:, dc, :],
                            start=(dc == 0),
                            stop=(dc == 7),
                        )
                    nc.scalar.activation(
                        out=hid[:, ff, :], in_=ps, func=AF.Relu,
                        bias=b1s[:, ff : ff + 1],
                    )
        with tc.psum_pool(name="ps_y", bufs=1) as psy:
            yaccs = [
                psy.tile([P, RQ], F32, tag=f"y{dy}", name=f"yacc{dy}")
                for dy in range(8)
            ]
            for q2 in range(4):  # W2 ffc-quarters [128, 8, 1024]
                w2q = wsp.tile([P, 8, 8 * P], F32R, tag="wbig")
                for j in range(8):
                    nc.sync.dma_start(
                        out=w2q[:, j, :], in_=io["w2T"][ds((q2 * 8 + j) * P, P), :]
                    )
                for dy in range(8):
                    for fc in range(8):
                        nc.tensor.matmul(
                            yaccs[dy],
                            lhsT=w2q[:, fc, ds(dy * P, P)],
                            rhs=hid[:, q2 * 8 + fc, :],
                            start=(q2 == 0 and fc == 0),
                            stop=(q2 == 3 and fc == 7),
                        )
            for dy in range(8):
                ysb = fsm.tile([P, RQ], F32, tag="ysb")
                nc.scalar.activation(
                    out=ysb, in_=yaccs[dy], func=AF.Identity,
                    bias=b2s[:, dy : dy + 1],
                )
                osb = fsm.tile([P, RQ], F32, tag="osb")
                nc.vector.tensor_add(out=osb, in0=ysb, in1=h1T[:, dy, :])
                nc.sync.dma_start(out=io["outT"][ds(dy * P, P), :], in_=osb)


def _build():
    nc = bacc.Bacc(
        "TRN2", target_bir_lowering=False, debug=False, num_devices=NCORES
    )
    io = {}
    def inp(name, shape, dt=F32):
        io[name] = nc.dram_tensor(name, shape, dt, kind="ExternalInput").ap()
    inp("xT", [D, N], F32R)
    inp("xTloc", [D, RQ], F32R)
    inp("xloc", [RQ, D])
    inp("wqT", [D, 2 * D], F32R)
    inp("wkT", [D, D], F32R)
    inp("wvT", [D, D], F32R)
    inp("w1T", [D, FF], F32R)
    inp("w2T", [FF, D], F32R)
    inp("bq2d", [P, 16])
    inp("bk2d", [P, 8])
    inp("bv2d", [P, 8])
    inp("b12d", [P, 32])
    inp("b22d", [P, 8])
    io["outT"] = nc.dram_tensor("outT", [D, RQ], F32, kind="ExternalOutput").ap()
    with tile.TileContext(nc) as tc:
        _tile_kernel(tc, io)
    nc.compile()
    return nc


_CACHE = {}


def _get_nc():
    if "nc" not in _CACHE:
        _CACHE["nc"] = _build()
    return _CACHE["nc"]


# local row permutation: row' p holds original local row 16*(p%16) + p//16,
# so kv-head group i = p//16 is 16 contiguous columns in qTp.
_PERM = np.array([16 * (p % 16) + p // 16 for p in range(RQ)])


def make_in_maps(inputs):
    x = np.ascontiguousarray(np.asarray(inputs["x"], np.float32)[0])
    xT = np.ascontiguousarray(x.T)

    def b2d(b, k):
        return np.ascontiguousarray(np.asarray(b, np.float32).reshape(k, P).T)

    wqT8 = (np.asarray(inputs["Wq"], np.float32) / 8.0).T  # [in, out]
    dup = (np.arange(16)[:, None] * 64 + (np.arange(P) % 64)[None, :]).ravel()
    bq8 = np.asarray(inputs["bq"], np.float32) / 8.0
    common = {
        "xT": xT,
        "wqT": np.ascontiguousarray(wqT8[:, dup]),  # [1024, 2048] duplicated
        "wkT": np.ascontiguousarray(np.asarray(inputs["Wk"], np.float32).T),
        "wvT": np.ascontiguousarray(np.asarray(inputs["Wv"], np.float32).T),
        "w1T": np.ascontiguousarray(np.asarray(inputs["W1"], np.float32).T),
        "w2T": np.ascontiguousarray(np.asarray(inputs["W2"], np.float32).T),
        "bq2d": np.ascontiguousarray(
            bq8[(np.arange(16)[None, :] * 64 + (np.arange(P) % 64)[:, None])]
        ),  # [128, 16]
        "bk2d": b2d(inputs["bk"], 8),
        "bv2d": b2d(inputs["bv"], 8),
        "b12d": b2d(inputs["b1"], 32),
        "b22d": b2d(inputs["b2"], 8),
    }
    in_maps = []
    for c in range(NCORES):
        rows = c * RQ + _PERM
        m = dict(common)
        m["xTloc"] = np.ascontiguousarray(xT[:, rows])
        m["xloc"] = np.ascontiguousarray(x[rows])
        in_maps.append(m)
    return in_maps


def kernel(**inputs):
    nc = _get_nc()
    res = run_bass_kernel_spmd(nc, make_in_maps(inputs), core_ids=list(range(NCORES)))
    out = np.empty((1, N, D), np.float32)
    for c in range(NCORES):
        out[0, c * RQ + _PERM, :] = res.results[c]["outT"].T
    return out



# revision 3
# speedup vs baseline: 1.1182x; 1.1182x over previous
"""Blockwise-parallel transformer layer on 8 TRN2 NeuronCores.

Sharding: by kv-head (the reference's einsum ties kv-head to seq pos mod 16).
Core c owns heads {2c, 2c+1} and the 256 seq rows n with n%16 in {2c, 2c+1}.
K/V projections therefore only need the 128-wide Wk/Wv column slice for the
core's two heads (8x less replicated GEMM work than seq-sharding, and no
collectives).  Scores run twice: q-major for the per-block max (DVE
reduce_max), kc-major for exp/num so no e-transposes are needed; the block
max is broadcast into the kc-major psum with an indicator matmul, and den
falls out of a ones-column augmented into V.

Shapes (hardcoded): x (1, 2048, 1024); Wq/Wk/Wv (1024, 1024); W1 (4096,
1024); W2 (1024, 4096); H=16 heads * HD=64; KB=16 kv blocks of 128.
"""

import sys
from contextlib import ExitStack

import numpy as np

for _p in ("/opt/trn_rl_repo", "/root/.axon_site/_ro/trn_rl_repo"):
    if _p not in sys.path:
        sys.path.append(_p)

import concourse.bass as bass  # noqa: E402
import concourse.tile as tile  # noqa: E402
from concourse import bacc, mybir  # noqa: E402
from concourse._compat import with_exitstack  # noqa: E402
from concourse.bass import ds  # noqa: E402
from concourse.bass_utils import run_bass_kernel_spmd  # noqa: E402
from concourse.masks import make_identity  # noqa: E402

D = 1024
H = 16
HD = 64
FF = 4096
N = 2048
KB = 16
NCORES = 8
RQ = N // NCORES  # 256 local rows
P = 128

F32 = mybir.dt.float32
F32R = mybir.dt.float32r
AX = mybir.AxisListType
AF = mybir.ActivationFunctionType


@with_exitstack
def _tile_kernel(ctx: ExitStack, tc: tile.TileContext, io: dict):
    nc = tc.nc

    consts = ctx.enter_context(tc.tile_pool(name="consts", bufs=1))
    ident = consts.tile([P, P], F32)
    make_identity(nc, ident)
    identr = consts.tile([P, P], F32R)
    nc.scalar.activation(out=identr, in_=ident, func=AF.Copy)
    # IND[j, kc] = 1 iff j == kc//128 (kc < 2048) — block-max broadcast
    tmpp = ctx.enter_context(tc.tile_pool(name="tmp_ind", bufs=1))
    indf = tmpp.tile([KB, KB * P], F32)
    nc.gpsimd.memset(indf, 1.0)
    nc.gpsimd.affine_select(
        out=indf, in_=indf, compare_op=mybir.AluOpType.is_ge, fill=0.0,
        base=0, pattern=[[1, KB * P]], channel_multiplier=-P,
    )
    nc.gpsimd.affine_select(
        out=indf, in_=indf, compare_op=mybir.AluOpType.is_ge, fill=0.0,
        base=P - 1, pattern=[[-1, KB * P]], channel_multiplier=P,
    )
    indc = consts.tile([KB, KB * P], F32R)
    nc.scalar.activation(out=indc, in_=indf, func=AF.Copy)
    bqs = consts.tile([HD, 16], F32)
    bks = consts.tile([HD, 2], F32)
    bvs = consts.tile([HD, 2], F32)
    b1s = consts.tile([P, 32], F32)
    b2s = consts.tile([P, 8], F32)
    nc.sync.dma_start(out=bqs, in_=io["bq2d"])
    nc.sync.dma_start(out=bks, in_=io["bk2d"])
    nc.sync.dma_start(out=bvs, in_=io["bv2d"])
    nc.sync.dma_start(out=b1s, in_=io["b12d"])
    nc.sync.dma_start(out=b2s, in_=io["b22d"])

    persist = ctx.enter_context(tc.tile_pool(name="persist", bufs=1))
    attn = persist.tile([P, 2, D], F32)  # [Q, h, (g,f)]

    with tc.tile_pool(name="kvq", bufs=1) as kvp:
        # kT2[f, h, n] = k[n, 64*(2c+h)+f];  vaug[n%128, K, h, f(+den)]
        kT2 = kvp.tile([HD, 2, N], F32R)
        vaug = kvp.tile([P, KB, 2, HD + 1], F32R)
        qT = kvp.tile([HD, 2, 16, P], F32R)  # [f, h, g, Q]
        nm = kvp.tile([P, 2, 16, KB], F32)  # -max per [Q, h, g, K]
        nmT = kvp.tile([KB, 2, 16, P], F32R)  # [K, h, (g,Q)]
        _kvq_proj(tc, io, kvp, identr, bqs, bks, bvs, kT2, vaug, qT)
        _pass_a(tc, kvp, ident, kT2, qT, nm, nmT)
        _pass_b(tc, kvp, ident, indc, kT2, vaug, qT, nmT, attn)

    _ffn_phase(tc, io, ident, b1s, b2s, attn)


def _kvq_proj(tc, io, kvp, identr, bqs, bks, bvs, kT2, vaug, qT):
    nc = tc.nc
    NCH = 4
    CW = N // NCH  # 512
    vT2 = kvp.tile([HD, 2, N], F32R)
    ones32 = kvp.tile([P, KB * 2], F32)
    nc.gpsimd.memset(ones32, 1.0)
    # den ones column at f=64 of every (K, h) slot
    nc.scalar.activation(
        out=vaug[:, :, :, HD : HD + 1].rearrange("p a b c -> p (a b c)"),
        in_=ones32,
        func=AF.Copy,
    )

    # K/V projections: only this core's 2-head dout slice (128 cols)
    with (
        tc.tile_pool(name="wkv", bufs=1) as wp,
        tc.tile_pool(name="xs", bufs=2) as xsp,
        tc.psum_pool(name="ps_kv", bufs=2) as pskv,
    ):
        wk = wp.tile([P, 8, P], F32R)
        wv = wp.tile([P, 8, P], F32R)
        for j in range(8):
            nc.sync.dma_start(out=wk[:, j, :], in_=io["wkT"][ds(j * P, P), :])
            nc.sync.dma_start(out=wv[:, j, :], in_=io["wvT"][ds(j * P, P), :])
        for c in range(NCH):
            xc = xsp.tile([P, 8, CW], F32R, tag="xc")
            for j in range(8):
                nc.sync.dma_start(
                    out=xc[:, j, :], in_=io["xT"][ds(j * P, P), ds(c * CW, CW)]
                )
            for w, bias_t, dst in ((wk, bks, kT2), (wv, bvs, vT2)):
                ps = pskv.tile([P, CW], F32, tag="pkv")
                for dx in range(8):
                    nc.tensor.matmul(
                        ps,
                        lhsT=w[:, dx, :],
                        rhs=xc[:, dx, :],
                        start=(dx == 0),
                        stop=(dx == 7),
                    )
                for h in range(2):
                    nc.scalar.activation(
                        out=dst[:, h, ds(c * CW, CW)],
                        in_=ps[ds(h * HD, HD), :],
                        func=AF.Identity,
                        bias=bias_t[:, h : h + 1],
                    )

    # vT2 -> vaug (n-major) via PE transposes
    with tc.psum_pool(name="ps_vt", bufs=2) as psvt:
        for h in range(2):
            for K in range(KB):
                vt = psvt.tile([P, HD], F32R, tag="vt")
                nc.tensor.transpose(
                    vt, vT2[:, h, ds(K * P, P)], identr[0:HD, 0:HD]
                )
                nc.vector.tensor_copy(out=vaug[:, K, h, 0:HD], in_=vt)

    # Q projection (g-pairs; dout 128 at a time), pre-scaled 1/8 host-side
    with (
        tc.tile_pool(name="wq", bufs=2) as wqp,
        tc.tile_pool(name="xtl", bufs=1) as xtlp,
        tc.psum_pool(name="ps_q", bufs=2) as psq,
    ):
        xtl = xtlp.tile([P, 8, RQ], F32R)
        for j in range(8):
            nc.sync.dma_start(out=xtl[:, j, :], in_=io["xTloc"][ds(j * P, P), :])
        for half in range(2):
            wq = wqp.tile([P, 8, 4 * P], F32R, tag="wq")
            for j in range(8):
                nc.sync.dma_start(
                    out=wq[:, j, :],
                    in_=io["wqT"][ds(j * P, P), ds(half * 4 * P, 4 * P)],
                )
            for t in range(4):
                gp = half * 4 + t  # g-pair index; g = 2*gp, 2*gp+1
                ps = psq.tile([P, RQ], F32, tag="pq")
                for dx in range(8):
                    nc.tensor.matmul(
                        ps,
                        lhsT=wq[:, dx, ds(t * P, P)],
                        rhs=xtl[:, dx, :],
                        start=(dx == 0),
                        stop=(dx == 7),
                    )
                for gh in range(2):
                    g = 2 * gp + gh
                    nc.scalar.activation(
                        out=qT[:, :, g, :],
                        in_=ps[ds(gh * HD, HD), :].rearrange(
                            "p (h q) -> p h q", h=2
                        ),
                        func=AF.Identity,
                        bias=bqs[:, g : g + 1],
                    )


def _pass_a(tc, kvp, ident, kT2, qT, nm, nmT):
    # q-major scores -> per-(q, kv-block) -max via DVE reduce, then transpose
    nc = tc.nc
    with (
        tc.psum_pool(name="ps_a", bufs=2) as psa,
        tc.psum_pool(name="ps_nt", bufs=2) as psnt,
    ):
        for h in range(2):
            for g in range(16):
                for sh in range(2):  # kc halves of 1024
                    s = psa.tile([P, 8 * P], F32, tag="s")
                    for c2 in range(2):
                        nc.tensor.matmul(
                            s[:, ds(c2 * 512, 512)],
                            lhsT=qT[:, h, g, :],
                            rhs=kT2[:, h, ds(sh * 1024 + c2 * 512, 512)],
                            start=True,
                            stop=True,
                        )
                    nc.vector.reduce_max(
                        out=nm[:, h, g, ds(sh * 8, 8)],
                        in_=s.rearrange("p (b f) -> p b f", f=P),
                        axis=AX.X,
                        negate=True,
                    )
                nt = psnt.tile([KB, P], F32, tag="nt")
                nc.tensor.transpose(nt, nm[:, h, g, :], ident)
                nc.vector.tensor_copy(out=nmT[:, h, g, :], in_=nt)


def _pass_b(tc, kvp, ident, indc, kT2, vaug, qT, nmT, attn):
    # kc-major: sT = k.q + broadcast(-max) -> exp -> num/den matmul
    nc = tc.nc
    nsb = kvp.tile([HD + 1, 2, N], F32)  # [f(+den), h, (g,Q)]
    with (
        tc.tile_pool(name="et", bufs=2) as etp,
        tc.psum_pool(name="ps_st", bufs=2) as psst,
        tc.psum_pool(name="ps_n", bufs=1) as psn,
    ):
        for h in range(2):
            nacc = psn.tile([HD + 1, N], F32, tag="nacc")
            for K in range(KB):
                et = etp.tile([P, 2, 8 * P], F32R, tag="et")
                for qh in range(2):  # q halves of 1024 (8 g each)
                    st = psst.tile([P, 8 * P], F32, tag="st")
                    for c2 in range(2):
                        sl = ds(c2 * 512, 512)
                        nc.tensor.matmul(
                            st[:, sl],
                            lhsT=kT2[:, h, ds(K * P, P)],
                            rhs=qT[:, h, ds(qh * 8 + c2 * 4, 4), :],
                            start=True,
                            stop=False,
                        )
                        nc.tensor.matmul(
                            st[:, sl],
                            lhsT=indc[:, ds(K * P, P)],
                            rhs=nmT[:, h, ds(qh * 8 + c2 * 4, 4), :],
                            start=False,
                            stop=True,
                            skip_group_check=True,
                        )
                    nc.scalar.activation(out=et[:, qh, :], in_=st, func=AF.Exp)
                for qh in range(2):
                    for c2 in range(2):
                        nc.tensor.matmul(
                            nacc[:, ds(qh * 1024 + c2 * 512, 512)],
                            lhsT=vaug[:, K, h, :],
                            rhs=et[:, qh, ds(c2 * 512, 512)],
                            start=(K == 0),
                            stop=(K == KB - 1),
                        )
            nc.scalar.activation(out=nsb[:, h, :], in_=nacc, func=AF.Identity)

    # finalize: transpose back to Q-partitions, scale by 1/den
    with (
        tc.tile_pool(name="fin", bufs=3) as finp,
        tc.psum_pool(name="ps_tr", bufs=2) as pstr,
    ):
        for h in range(2):
            for g in range(16):
                tr = pstr.tile([P, HD + 1], F32, tag="tr")
                nc.tensor.transpose(
                    tr, nsb[:, h, ds(g * P, P)], ident[0 : HD + 1, 0 : HD + 1]
                )
                rcp = finp.tile([P, 1], F32, tag="rcp")
                nc.vector.reciprocal(out=rcp, in_=tr[:, HD : HD + 1])
                nc.vector.tensor_scalar_mul(
                    out=attn[:, h, ds(g * HD, HD)], in0=tr[:, 0:HD], scalar1=rcp
                )


def _ffn_phase(tc, io, ident, b1s, b2s, attn):
    nc = tc.nc
    with (
        tc.tile_pool(name="ffn", bufs=1) as fp,
        tc.tile_pool(name="ffn_sm", bufs=3) as fsm,
        tc.tile_pool(name="wstream", bufs=2) as wsp,
    ):
        xl = fp.tile([P, 2, D], F32)  # local x rows (residual), [Q, h, d]
        nc.sync.dma_start(out=xl[:, 0, :], in_=io["xloc"][0:P, :])
        nc.sync.dma_start(out=xl[:, 1, :], in_=io["xloc"][P : 2 * P, :])
        h1 = fp.tile([P, 2, D], F32)
        for qt in range(2):
            nc.vector.tensor_add(out=h1[:, qt, :], in0=attn[:, qt, :], in1=xl[:, qt, :])
        h1T = fp.tile([P, 8, RQ], F32)
        h1Tr = fp.tile([P, 8, RQ], F32R)  # f32r copy for GEMM1 rhs
        with tc.psum_pool(name="ps_ft", bufs=2) as psft:
            for qt in range(2):
                for dc in range(8):
                    tps = psft.tile([P, P], F32, tag="tps")
                    nc.tensor.transpose(tps, h1[:, qt, ds(dc * P, P)], ident)
                    nc.vector.tensor_copy(out=h1T[:, dc, ds(qt * P, P)], in_=tps)
                    nc.scalar.activation(
                        out=h1Tr[:, dc, ds(qt * P, P)], in_=tps, func=AF.Copy
                    )
        hid = fp.tile([P, 32, RQ], F32R)
        with tc.psum_pool(name="ps_f", bufs=3) as psf:
            for q4 in range(4):  # W1 column quarters [128, 8, 1024]
                w1q = wsp.tile([P, 8, 8 * P], F32R, tag="wbig")
                for j in range(8):
                    nc.sync.dma_start(
                        out=w1q[:, j, :],
                        in_=io["w1T"][ds(j * P, P), ds(q4 * 8 * P, 8 * P)],
                    )
                for f in range(8):
                    ff = q4 * 8 + f
                    ps = psf.tile([P, RQ], F32, tag="fps")
                    for dc in range(8):
                        nc.tensor.matmul(
                            ps,
                            lhsT=w1q[:, dc, ds(f * P, P)],
                            rhs=h1Tr[:, dc, :],
                            start=(dc == 0),
                            stop=(dc == 7),
                        )
                    nc.scalar.activation(
                        out=hid[:, ff, :], in_=ps, func=AF.Relu,
                        bias=b1s[:, ff : ff + 1],
                    )
        with tc.psum_pool(name="ps_y", bufs=1) as psy:
            yaccs = [
                psy.tile([P, RQ], F32, tag=f"y{dy}", name=f"yacc{dy}")
                for dy in range(8)
            ]
            for q2 in range(4):  # W2 ffc-quarters [128, 8, 1024]
                w2q = wsp.tile([P, 8, 8 * P], F32R, tag="wbig")
                for j in range(8):
                    nc.sync.dma_start(
                        out=w2q[:, j, :], in_=io["w2T"][ds((q2 * 8 + j) * P, P), :]
                    )
                for dy in range(8):
                    for fc in range(8):
                        nc.tensor.matmul(
                            yaccs[dy],
                            lhsT=w2q[:, fc, ds(dy * P, P)],
                            rhs=hid[:, q2 * 8 + fc, :],
                            start=(q2 == 0 and fc == 0),
                            stop=(q2 == 3 and fc == 7),
                        )
            for dy in range(8):
                ysb = fsm.tile([P, RQ], F32, tag="ysb")
                nc.scalar.activation(
                    out=ysb, in_=yaccs[dy], func=AF.Identity,
                    bias=b2s[:, dy : dy + 1],
                )
                osb = fsm.tile([P, RQ], F32, tag="osb")
                nc.vector.tensor_add(out=osb, in0=ysb, in1=h1T[:, dy, :])
                nc.sync.dma_start(out=io["outT"][ds(dy * P, P), :], in_=osb)


def _build():
    nc = bacc.Bacc(
        "TRN2", target_bir_lowering=False, debug=False, num_devices=NCORES
    )
    io = {}
    def inp(name, shape, dt=F32):
        io[name] = nc.dram_tensor(name, shape, dt, kind="ExternalInput").ap()
    inp("xT", [D, N], F32R)
    inp("xTloc", [D, RQ], F32R)
    inp("xloc", [RQ, D])
    inp("wqT", [D, D], F32R)
    inp("wkT", [D, P], F32R)
    inp("wvT", [D, P], F32R)
    inp("w1T", [D, FF], F32R)
    inp("w2T", [FF, D], F32R)
    inp("bq2d", [HD, 16])
    inp("bk2d", [HD, 2])
    inp("bv2d", [HD, 2])
    inp("b12d", [P, 32])
    inp("b22d", [P, 8])
    io["outT"] = nc.dram_tensor("outT", [D, RQ], F32, kind="ExternalOutput").ap()
    with tile.TileContext(nc) as tc:
        _tile_kernel(tc, io)
    nc.compile()
    return nc


_CACHE = {}


def _get_nc():
    if "nc" not in _CACHE:
        _CACHE["nc"] = _build()
    return _CACHE["nc"]


def _rows_for_core(c):
    # local row r = h*128 + Q  ->  global n = Q*16 + 2c + h
    r = np.arange(RQ)
    h, Q = r // P, r % P
    return Q * 16 + 2 * c + h


def make_in_maps(inputs):
    x = np.ascontiguousarray(np.asarray(inputs["x"], np.float32)[0])
    xT = np.ascontiguousarray(x.T)

    wqT8 = np.ascontiguousarray((np.asarray(inputs["Wq"], np.float32) / 8.0).T)
    bq8 = np.asarray(inputs["bq"], np.float32) / 8.0
    wkT = np.asarray(inputs["Wk"], np.float32).T  # [din, dout]
    wvT = np.asarray(inputs["Wv"], np.float32).T
    bk = np.asarray(inputs["bk"], np.float32)
    bv = np.asarray(inputs["bv"], np.float32)

    def b2d(b, k):
        return np.ascontiguousarray(np.asarray(b, np.float32).reshape(k, P).T)

    common = {
        "xT": xT,
        "wqT": wqT8,
        "bq2d": np.ascontiguousarray(bq8.reshape(16, HD).T),  # [f, g]
        "w1T": np.ascontiguousarray(np.asarray(inputs["W1"], np.float32).T),
        "w2T": np.ascontiguousarray(np.asarray(inputs["W2"], np.float32).T),
        "b12d": b2d(inputs["b1"], 32),
        "b22d": b2d(inputs["b2"], 8),
    }
    in_maps = []
    for c in range(NCORES):
        rows = _rows_for_core(c)
        sl = slice(c * P, (c + 1) * P)
        m = dict(common)
        m["xTloc"] = np.ascontiguousarray(xT[:, rows])
        m["xloc"] = np.ascontiguousarray(x[rows])
        m["wkT"] = np.ascontiguousarray(wkT[:, sl])
        m["wvT"] = np.ascontiguousarray(wvT[:, sl])
        m["bk2d"] = np.ascontiguousarray(bk[sl].reshape(2, HD).T)
        m["bv2d"] = np.ascontiguousarray(bv[sl].reshape(2, HD).T)
        in_maps.append(m)
    return in_maps


def kernel(**inputs):
    nc = _get_nc()
    res = run_bass_kernel_spmd(nc, make_in_maps(inputs), core_ids=list(range(NCORES)))
    out = np.empty((1, N, D), np.float32)
    for c in range(NCORES):
        out[0, _rows_for_core(c), :] = res.results[c]["outT"].T
    return out
